# revision 27
# baseline (speedup 1.0000x reference)
"""Trainium2 Bass kernel for the ConvModule problem (DFT8 conv version).

Computes, for x (B=16, T=1024, C=512) fp32:
    h = LayerNorm_C(x) -> pw conv C->2C + Swish -> k=5 conv 2C->2C
      -> GLU -> BatchNorm(eval) -> pw conv C->C
Data-parallel over batch across 8 NeuronCores (2 batches/core, weights
replicated).  LN gamma/beta folded into w1/b1, BN folded into w3/b3 on the
host.

The k=5 'same' conv is a length-8 cyclic correlation per tile of 4 outputs
(exact since max tap reach 3+4 <= 7): a real FFT8 on the device with
host-transformed weights U = conj(FFT8(pad(w2)))/8.  14 GEMM passes per 4
outputs instead of the direct method's 20.

v3 notes:
- host t-permutation (block row 4u+j -> partition j*32+u) makes the conv1
  Swish deinterleave and the GLU re-interleave inner-contiguous APs.
- Swish planes are fp32 with zero-padded edge columns; the FFT first stage
  reads +-1-shifted 4B-aligned slices (no halo copies).
- All elementwise work on VectorE (gpsimd compute contends for the shared
  SBUF port and slows DVE ~1.5x); PSUM evacuation on ScalarE (Copy needs no
  act table).  Butterfly ops are pair-fused into [P,2,256] ops wherever the
  two halves share the same ALU op.
- conv3 bias b3 enters as a K=1 GEMM pass (ones row x b3 row), so the PSUM
  result only needs a scalar copy before the store.
"""

from contextlib import ExitStack

import numpy as np

import concourse.bass as bass
import concourse.bacc as bacc
import concourse.tile as tile
from concourse import mybir
from concourse.masks import make_identity
from concourse.bass_utils import run_bass_kernel_spmd

B, T, C, K = 16, 1024, 512, 5
EPS_LN = 1e-5
EPS_BN = 1e-5
NCORES = 8
BLOC = B // NCORES          # batches per core
P = 128                     # SBUF partitions
CB = C // P                 # 4 channel blocks of the C dim
OB = (2 * C) // P           # 8 channel blocks of the 2C dim
TH = T // 2                 # 512
NT = T // 4                 # 256 conv tiles per batch (4 outputs each)
NU = 11                     # stored U planes: U0, U4, (P1,P2,P3) x j=1..3
W = NT + 4                  # padded plane width, data at cols 2..257
F32 = mybir.dt.float32
BF16 = mybir.dt.bfloat16
RS2 = float(1.0 / np.sqrt(2.0))

AF = mybir.ActivationFunctionType
ALU = mybir.AluOpType

# GEMM pass lists: (psum plane index, [(u_idx, v_name), ...])
# u planes: 0:U0 1:U4, then per j in 1..3 at 2+3(j-1): P1=2Re, P2=2Im, P3=-2Re
# E group: M0, M4, M2r, M2i ; O group: M1r, M1i, M3r, M3i
MPASS_E = [
    (0, [(0, "v0")]),
    (1, [(1, "v4")]),
    (2, [(5, "v2r"), (6, "v2if")]),
    (3, [(6, "v2r"), (7, "v2if")]),
]
MPASS_O = [
    (0, [(2, "v1r"), (3, "v1if")]),
    (1, [(3, "v1r"), (4, "v1if")]),
    (2, [(8, "v3r"), (9, "v3if")]),
    (3, [(9, "v3r"), (10, "v3if")]),
]
# V planes live in pair tiles [P, 2, BLOC, NT]: (pair key, slot)
VSLOT = {"v0": ("p0", 0), "v4": ("p0", 1), "v1r": ("p1", 0), "v1if": ("p1", 1),
         "v2r": ("p2", 0), "v2if": ("p2", 1), "v3r": ("p3", 0), "v3if": ("p3", 1)}


def build_nc() -> bass.Bass:
    nc = bacc.Bacc("TRN2")

    xs = nc.declare_dram_parameter("xs", [BLOC, T, C], BF16, isOutput=False)
    w1t = nc.declare_dram_parameter("w1t", [CB, P, 2 * C], BF16, isOutput=False)
    ut = nc.declare_dram_parameter("ut", [OB, P, NU, OB, P], BF16, isOutput=False)
    w3t = nc.declare_dram_parameter("w3t", [CB, P, C], BF16, isOutput=False)
    b1 = nc.declare_dram_parameter("b1", [P, OB], F32, isOutput=False)
    b2 = nc.declare_dram_parameter("b2", [P, OB], F32, isOutput=False)
    b3 = nc.declare_dram_parameter("b3", [1, C], BF16, isOutput=False)
    out = nc.declare_dram_parameter("out", [BLOC, T, C], F32, isOutput=True)

    with ExitStack() as ctx:
        tc = ctx.enter_context(tile.TileContext(nc))

        consts = ctx.enter_context(tc.tile_pool(name="consts", bufs=1))
        xin = ctx.enter_context(tc.tile_pool(name="xin", bufs=3))
        stats = ctx.enter_context(tc.tile_pool(name="stats", bufs=4))
        hNp = ctx.enter_context(tc.tile_pool(name="hNp", bufs=1))
        h1rp = ctx.enter_context(tc.tile_pool(name="h1rp", bufs=3))
        fsc = ctx.enter_context(tc.tile_pool(name="fsc", bufs=1))
        vpool = ctx.enter_context(tc.tile_pool(name="vpool", bufs=1))
        upool = ctx.enter_context(tc.tile_pool(name="upool", bufs=2))
        mcp = ctx.enter_context(tc.tile_pool(name="mcp", bufs=2))
        isc = ctx.enter_context(tc.tile_pool(name="isc", bufs=1))
        yap = ctx.enter_context(tc.tile_pool(name="yap", bufs=2))
        hGp = ctx.enter_context(tc.tile_pool(name="hGp", bufs=1))
        outp = ctx.enter_context(tc.tile_pool(name="outp", bufs=2))
        cv_psum = ctx.enter_context(tc.tile_pool(name="cv_psum", bufs=2, space="PSUM"))
        ab_psum = ctx.enter_context(tc.tile_pool(name="ab_psum", bufs=2, space="PSUM"))
        o_psum = ctx.enter_context(tc.tile_pool(name="o_psum", bufs=2, space="PSUM"))

        # ---- constants / persistent weights ----
        ident = consts.tile([P, P], BF16, tag="ident")
        make_identity(nc, ident)
        epssb = consts.tile([P, 1], F32, tag="eps")
        nc.vector.memset(epssb, EPS_LN)
        invC = consts.tile([P, 1], BF16, tag="invC")
        nc.vector.memset(invC, 1.0 / C)
        ones1 = consts.tile([1, P], BF16, tag="ones1")
        nc.vector.memset(ones1, 1.0)
        b1sb = consts.tile([P, OB], F32, tag="b1")
        nc.sync.dma_start(out=b1sb, in_=b1[:])
        b2sb = consts.tile([P, OB], F32, tag="b2")
        nc.sync.dma_start(out=b2sb, in_=b2[:])
        b3sb = consts.tile([1, C], BF16, tag="b3")
        nc.sync.dma_start(out=b3sb, in_=b3[:])
        w1sb = []
        for cb in range(CB):
            w = consts.tile([P, 2 * C], BF16, tag=f"w1_{cb}", name=f"w1_{cb}")
            nc.sync.dma_start(out=w, in_=w1t[cb])
            w1sb.append(w)
        w3sb = []
        for cb in range(CB):
            w = consts.tile([P, C], BF16, tag=f"w3_{cb}", name=f"w3_{cb}")
            nc.sync.dma_start(out=w, in_=w3t[cb])
            w3sb.append(w)

        # V planes: pair tiles [P, 2, BLOC, NT] per (pair, ib)
        vsb = {}
        for pk in ("p0", "p1", "p2", "p3"):
            for ib in range(OB):
                v = vpool.tile([P, 2, BLOC, NT], BF16, tag=f"{pk}_{ib}",
                               name=f"{pk}_{ib}")
                vsb[(pk, ib)] = v

        def vslot(vn, ib, b):
            pk, sl = VSLOT[vn]
            return vsb[(pk, ib)][:, sl, b, :]

        hg_tiles = {}
        hN_all = {}

        # prefetch the first two U slabs before any other DMA-heavy phase
        uslabs = {}
        for ob in (0, CB):
            usb = upool.tile([P, NU, OB, P], BF16, tag="uslab",
                             name=f"uslab_{ob}_pre")
            nc.sync.dma_start(out=usb, in_=ut[ob])
            uslabs[ob] = usb

        # ---------- Phase A: transpose raw x, LN via matmul stats ----------
        # Transposes run on RAW x (no stats wait).  Per T-half: mean and
        # E[x^2] come from K=C matmuls against a 1/C column (PSUM rows),
        # rows are partition-broadcast, rstd = Rsqrt(var+eps) on ScalarE,
        # then hN is normalized half-by-half with two DVE ops per cb block.
        def phase_A(b, psp):
            xsr = xs[b].rearrange("(tb p) c -> tb p c", p=P)
            hN = hNp.tile([P, CB * T], BF16, tag=f"hN{b}", name=f"hN_{b}")
            hN_all[b] = hN
            hN3 = hN[:, :].rearrange("p (c t) -> p c t", c=CB)
            for ph in range(2):
                for tb in range(4 * ph, 4 * ph + 4):
                    xb = xin.tile([P, C], BF16, tag=f"xbig{b}",
                                  name=f"xbig_{b}_{tb}")
                    nc.gpsimd.dma_start(out=xb, in_=xsr[tb])
                    ps = psp.tile([P, TH], BF16, tag="abp",
                                      name=f"tp_{b}_{tb}")
                    for cb in range(CB):
                        nc.tensor.transpose(
                            ps[:, cb * P:(cb + 1) * P],
                            xb[:, cb * P:(cb + 1) * P], ident)
                    nc.scalar.copy(
                        out=hN3[:, :, tb * P:(tb + 1) * P],
                        in_=ps[:, 0:CB * P].rearrange("p (c i) -> p c i", c=CB))
                half = hN3[:, :, ph * TH:(ph + 1) * TH]
                hsq = stats.tile([P, CB, TH], BF16, tag="hsq",
                                 name=f"hsq_{b}_{ph}", bufs=1)
                nc.scalar.activation(out=hsq, in_=half, func=AF.Square)
                mp = psp.tile([1, TH], F32, tag="abp", name=f"mp_{b}_{ph}")
                for cb in range(CB):
                    nc.tensor.matmul(mp, invC, half[:, cb, :],
                                     start=(cb == 0), stop=(cb == CB - 1))
                sp = psp.tile([1, TH], F32, tag="abp", name=f"sp_{b}_{ph}")
                for cb in range(CB):
                    nc.tensor.matmul(sp, invC, hsq[:, cb, :],
                                     start=(cb == 0), stop=(cb == CB - 1))
                msrow = stats.tile([1, 2, TH], BF16, tag="msrow", bufs=1)
                nc.scalar.copy(out=msrow[:, 0, :], in_=mp)
                nc.scalar.copy(out=msrow[:, 1, :], in_=sp)
                msB = stats.tile([P, 2, TH], BF16, tag="msB", bufs=1)
                nc.gpsimd.partition_broadcast(
                    msB.rearrange("p a t -> p (a t)"),
                    msrow.rearrange("p a t -> p (a t)"))
                meanB = msB[:, 0, :]
                vB = stats.tile([P, TH], BF16, tag="vB", bufs=1)
                nc.vector.tensor_mul(out=vB, in0=meanB, in1=meanB)
                nc.vector.tensor_sub(out=vB, in0=msB[:, 1, :], in1=vB)
                sB = psp.tile([P, TH], F32, tag="abp", name=f"sB_{b}_{ph}")
                nc.scalar.activation(out=sB, in_=vB, func=AF.Sqrt,
                                     bias=epssb, scale=1.0)
                rBf = psp.tile([P, TH], F32, tag="abp", name=f"rBf_{b}_{ph}")
                nc.vector.reciprocal_approx_fast(out=rBf, in_=sB)
                rB = stats.tile([P, TH], BF16, tag="rB", bufs=1)
                nc.vector.tensor_copy(rB, rBf)
                for cb in range(CB):
                    sl = half[:, cb, :]
                    tmpn = stats.tile([P, TH], BF16, tag="tmpn", bufs=1)
                    nc.vector.tensor_sub(out=tmpn, in0=sl, in1=meanB)
                    nc.vector.tensor_mul(out=sl, in0=tmpn, in1=rB)

        # ---------- FFT8 per (ib, batch): silu planes -> 8 V planes ----------
        # Swish planes Pj (j=0..3) sit at t_[:, j, 2:2+NT] fp32, zeros at
        # cols 0..1 and NT+2..NT+3.  FFT8 input d[i]:
        #   d[j+2] = Pj[.]; d0 = P2[.-1]; d1 = P3[.-1]; d6 = P0[.+1]; d7 = P1[.+1]
        # Butterfly with pair-fused ops:
        #   [s0,s1] = [P2,P3][.-1] + [P2,P3][.]   [s2,s3] = [P0,P1][.] + [P0,P1][.+1]
        #   [t0,t1] = [P2,P3][.-1] - [P2,P3][.]   [t2,t3] = [P0,P1][.] - [P0,P1][.+1]
        # t-tile layout [P, 2, 2, NT]: (x, par) with t0=(0,0) t2=(0,1) t1=(1,0) t3=(1,1)
        def fft_ib(t_, ib, b):
            def ftl(tag, shape):
                return fsc.tile(shape, BF16, tag=tag, name=f"{tag}_{ib}_{b}")

            # scalar materializes the +-1-shifted plane pairs at even offsets
            # so every stage-1 pair op runs in the DVE 2x packed mode
            shA = ftl("shA", [P, 2, NT])
            nc.scalar.copy(out=shA, in_=t_[:, 2:4, 1:1 + NT])   # P2,P3 [u-1]
            shB = ftl("shB", [P, 2, NT])
            nc.scalar.copy(out=shB, in_=t_[:, 0:2, 3:3 + NT])   # P0,P1 [u+1]
            hi = t_[:, 2:4, 2:2 + NT]
            lo = t_[:, 0:2, 2:2 + NT]
            st = ftl("s", [P, 4, NT])
            tt = ftl("t", [P, 2, 2, NT])
            nc.vector.tensor_add(out=st[:, 0:2, :], in0=shA, in1=hi)
            nc.vector.tensor_add(out=st[:, 2:4, :], in0=lo, in1=shB)
            nc.vector.tensor_sub(out=tt[:, :, 0, :], in0=shA, in1=hi)
            nc.vector.tensor_sub(out=tt[:, :, 1, :], in0=lo, in1=shB)
            ut_ = ftl("u", [P, 2, NT])
            # [u0,u1] = [s0,s1]+[s2,s3] ; V2 pair = [s0,s1]-[s2,s3]
            nc.vector.tensor_add(out=ut_, in0=st[:, 0:2, :], in1=st[:, 2:4, :])
            nc.vector.tensor_sub(out=vsb[("p2", ib)][:, :, b, :],
                                 in0=st[:, 0:2, :], in1=st[:, 2:4, :])
            nc.vector.tensor_add(out=vslot("v0", ib, b), in0=ut_[:, 0, :],
                                 in1=ut_[:, 1, :])
            nc.vector.tensor_sub(out=vslot("v4", ib, b), in0=ut_[:, 0, :],
                                 in1=ut_[:, 1, :])
            t1, t3 = tt[:, 1, 0, :], tt[:, 1, 1, :]
            a = ftl("fa", [P, NT])
            nc.vector.tensor_sub(out=a, in0=t1, in1=t3)
            bb = ftl("fb", [P, NT])
            nc.vector.tensor_add(out=bb, in0=t1, in1=t3)
            apbp = ftl("fab", [P, 2, NT])
            nc.scalar.activation(out=apbp[:, 0, :], in_=a, func=AF.Copy,
                                 scale=RS2)
            nc.scalar.activation(out=apbp[:, 1, :], in_=bb, func=AF.Copy,
                                 scale=RS2)
            # [V1r,V1if] = [t0,t2] + [ap,bp]
            nc.vector.tensor_add(out=vsb[("p1", ib)][:, :, b, :],
                                 in0=tt[:, 0, :, :], in1=apbp)
            nc.vector.tensor_sub(out=vslot("v3r", ib, b), in0=tt[:, 0, 0, :],
                                 in1=apbp[:, 0, :])
            nc.vector.tensor_sub(out=vslot("v3if", ib, b), in0=apbp[:, 1, :],
                                 in1=tt[:, 0, 1, :])

        # ---------- Phase B: conv1 C->2C + Swish into deinterleaved planes ----
        def phase_B(b, psp):
            hN3 = hN_all[b][:, :].rearrange("p (c t) -> p c t", c=CB)
            for ob in range(OB):
                t_ = h1rp.tile([P, 4, W], BF16, tag="h1r", name=f"h1r_{ob}_{b}")
                nc.vector.memset(t_[:, :, 0:2], 0.0)
                nc.vector.memset(t_[:, :, W - 2:W], 0.0)
                # Silu(z + b1) -> plane j col 2+128*ph+32*blk+u
                # (pz cols are (blk, j, u) thanks to the host permutation)
                for ph in range(2):
                    pz = psp.tile([P, TH], F32, tag="abp",
                                      name=f"pz_{ob}_{b}_{ph}")
                    for h2 in range(2):
                        cs = ph * TH + h2 * 256
                        for cb in range(CB):
                            w = w1sb[cb][:, ob * P:(ob + 1) * P]
                            nc.tensor.matmul(
                                pz[:, h2 * 256:(h2 + 1) * 256], w,
                                hN3[:, cb, cs:cs + 256],
                                start=(cb == 0), stop=(cb == CB - 1))
                    dst = t_[:, :, 2 + 128 * ph:2 + 128 * ph + 128]
                    nc.scalar.activation(
                        out=dst.rearrange("p j (blk u) -> p blk j u",
                                          blk=4, u=32),
                        in_=pz,
                        func=AF.Silu, bias=b1sb[:, ob:ob + 1], scale=1.0,
                    )
                fft_ib(t_, ob, b)

        # ---------- conv GEMM + IFFT per (ob, batch) ----------
        def gemm_group(mm, passes, b, usb):
            for pl, plist in passes:
                n = len(plist) * OB
                i = 0
                for (ui, vn) in plist:
                    for ib in range(OB):
                        nc.tensor.matmul(
                            mm[:, pl, :],
                            usb[:, ui, ib, :],
                            vslot(vn, ib, b),
                            start=(i == 0), stop=(i == n - 1),
                        )
                        i += 1

        def conv_ob(ob, b, usb):
            def tl(tag, shape=None):
                return isc.tile(shape or [P, NT], BF16, tag=tag,
                                name=f"{tag}_{ob}_{b}")

            mc = mcp.tile([P, 8, NT], BF16, tag="mc", name=f"mc_{ob}_{b}")
            # E group: M0, M4, M2r, M2i (x2 folded into U2 on host)
            mmE = cv_psum.tile([P, 4, NT], F32, tag="mm", name=f"mmE_{ob}_{b}")
            gemm_group(mmE, MPASS_E, b, usb)
            nc.scalar.copy(out=mc[:, 0:4, :], in_=mmE[:, :, :])
            ptq = tl("iPQ", [P, 2, NT])
            nc.vector.tensor_add(out=ptq[:, 0, :], in0=mc[:, 0, :], in1=mc[:, 1, :])
            nc.vector.tensor_sub(out=ptq[:, 1, :], in0=mc[:, 0, :], in1=mc[:, 1, :])
            # [E0,E3] = [Pt,Qt] + [M2r,M2i] ; [E2,E1] = [Pt,Qt] - [M2r,M2i]
            e03 = tl("iE03", [P, 2, NT])
            nc.vector.tensor_add(out=e03, in0=ptq, in1=mc[:, 2:4, :])
            e21 = tl("iE21", [P, 2, NT])
            nc.vector.tensor_sub(out=e21, in0=ptq, in1=mc[:, 2:4, :])
            # O group: M1r, M1i, M3r, M3i (x2 folded into U1/U3 on host)
            mmO = cv_psum.tile([P, 4, NT], F32, tag="mm", name=f"mmO_{ob}_{b}")
            gemm_group(mmO, MPASS_O, b, usb)
            nc.scalar.copy(out=mc[:, 4:8, :], in_=mmO[:, :, :])
            M1r, M1i = mc[:, 4, :], mc[:, 5, :]
            M3r, M3i = mc[:, 6, :], mc[:, 7, :]
            y = yap.tile([P, 4, NT], BF16, tag="ya" if ob < CB else "yg",
                         name=f"y_{ob}_{b}")
            w0 = tl("iw0")
            nc.vector.tensor_add(out=w0, in0=M1r, in1=M3r)
            nc.vector.tensor_add(out=y[:, 0, :], in0=e03[:, 0, :], in1=w0)
            ac = tl("iac", [P, 2, NT])      # [aa, cc] = M1r -+ M1i
            nc.vector.tensor_sub(out=ac[:, 0, :], in0=M1r, in1=M1i)
            nc.vector.tensor_add(out=ac[:, 1, :], in0=M1r, in1=M1i)
            bd = tl("ibd", [P, 2, NT])      # [bb2, dd] = M3r +- M3i
            nc.vector.tensor_add(out=bd[:, 0, :], in0=M3r, in1=M3i)
            nc.vector.tensor_sub(out=bd[:, 1, :], in0=M3r, in1=M3i)
            t13 = tl("it13", [P, 2, NT])    # [t1v, t3v] = [aa,cc] - [bb2,dd]
            nc.vector.tensor_sub(out=t13, in0=ac, in1=bd)
            nc.vector.scalar_tensor_tensor(
                out=y[:, 1, :], in0=t13[:, 0, :], scalar=RS2, in1=e21[:, 1, :],
                op0=ALU.mult, op1=ALU.add)
            w2v = tl("iw2")
            nc.vector.tensor_sub(out=w2v, in0=M3i, in1=M1i)
            nc.vector.tensor_add(out=y[:, 2, :], in0=e21[:, 0, :], in1=w2v)
            nc.vector.scalar_tensor_tensor(
                out=y[:, 3, :], in0=t13[:, 1, :], scalar=-RS2, in1=e03[:, 1, :],
                op0=ALU.mult, op1=ALU.add)
            return y

        # ---------- GLU per (value-ob v, batch) ----------
        def glu(v, b, ya, yg):
            sg = isc.tile([P, 4, NT], BF16, tag="sg", name=f"sg_{v}_{b}", bufs=1)
            nc.scalar.activation(
                out=sg.rearrange("p j u -> p (j u)"),
                in_=yg.rearrange("p j u -> p (j u)"),
                func=AF.Sigmoid, bias=b2sb[:, v + CB:v + CB + 1], scale=1.0,
            )
            # hg col tb*128 + j*32 + u = (ya[j, 32tb+u] + b2[v]) * sg[j, ...]
            # (permuted interleave; one 3D-out STT per j plane)
            hg = hGp.tile([P, T], BF16, tag=f"hg{v}", name=f"hg{v}_{b}")
            hg_tiles[(v, b)] = hg
            hg3 = hg.rearrange("p (tb q) -> p tb q", tb=8, q=P)
            for j in range(4):
                nc.vector.scalar_tensor_tensor(
                    out=hg3[:, :, 32 * j:32 * (j + 1)],
                    in0=ya[:, j, :], scalar=b2sb[:, v:v + 1], in1=sg[:, j, :],
                    op0=ALU.add, op1=ALU.mult,
                )

        # ---------- Phase D: conv3 with activations stationary ----------
        def phase_D(b):
            cbo = list(range(CB)) if b % 2 == 0 else list(reversed(range(CB)))
            for tb in range(T // P):
                po = o_psum.tile([P, C], F32, tag="abp", name=f"po_{b}_{tb}")
                # K=1 bias pass: po starts at ones^T @ b3 = broadcast b3
                nc.tensor.matmul(po, ones1[0:1, :], b3sb[0:1, :],
                                 start=True, stop=False)
                for k, cb in enumerate(cbo):
                    hg = hg_tiles[(cb, b)]
                    nc.tensor.matmul(
                        po, hg[:, P * tb:P * (tb + 1)], w3sb[cb],
                        start=False, stop=(k == CB - 1),
                    )
                obig = outp.tile([P, C], F32, tag="obig", name=f"ob_{b}_{tb}")
                nc.scalar.copy(out=obig, in_=po)
                nc.scalar.dma_start(
                    out=out[b].rearrange("(tb p) c -> p tb c", p=P)[:, tb, :],
                    in_=obig,
                )

        # ================= schedule =================
        phase_A(0, ab_psum)
        phase_A(1, o_psum)
        phase_B(0, ab_psum)
        phase_B(1, o_psum)
        FWD = [0, CB, 1, 1 + CB, 2, 2 + CB, 3, 3 + CB]
        last = [None, None]
        for b in range(BLOC):
            order = FWD if b % 2 == 0 else list(reversed(FWD))
            pend = {}
            for ob in order:
                if ob in uslabs:
                    usb = uslabs.pop(ob)
                    last = [ob, usb]
                elif last[0] == ob:
                    usb = last[1]
                else:
                    usb = upool.tile([P, NU, OB, P], BF16, tag="uslab",
                                     name=f"uslab_{ob}_{b}")
                    nc.sync.dma_start(out=usb, in_=ut[ob])
                    last = [ob, usb]
                y = conv_ob(ob, b, usb)
                v = ob % CB
                if v in pend:
                    pair = pend.pop(v)
                    ya, yg = (pair, y) if ob >= CB else (y, pair)
                    glu(v, b, ya, yg)
                else:
                    pend[v] = y
            phase_D(b)

    nc.compile()
    return nc


def prepare_inputs(x, ln_g, ln_b, w1, b1, w2, b2, bn_g, bn_b, bn_mean, bn_var, w3, b3):
    """Host-side folding + DFT weight transform + layout."""
    f = np.float32
    bf = mybir.dt.np(BF16)
    x = np.asarray(x, f)
    ln_g, ln_b = np.asarray(ln_g, f), np.asarray(ln_b, f)
    w1, b1 = np.asarray(w1, f), np.asarray(b1, f)
    w2, b2 = np.asarray(w2, f), np.asarray(b2, f)
    bn_g, bn_b = np.asarray(bn_g, f), np.asarray(bn_b, f)
    bn_mean, bn_var = np.asarray(bn_mean, f), np.asarray(bn_var, f)
    w3, b3 = np.asarray(w3, f), np.asarray(b3, f)

    # Fold LN affine into conv1, BN (eval) into conv3.
    w1f = w1 * ln_g[None, :]
    b1f = b1 + w1 @ ln_b
    s_bn = bn_g / np.sqrt(bn_var + EPS_BN)
    w3f = w3 * s_bn[None, :]
    b3f = b3 + w3 @ (bn_b - bn_mean * s_bn)

    w1d = np.ascontiguousarray(w1f.T.reshape(CB, P, 2 * C)).astype(bf)
    w3d = np.ascontiguousarray(w3f.T.reshape(CB, P, C)).astype(bf)

    # U planes: Uc = conj(FFT8(pad(w2)))/8, w2 is (K, I, O)
    wf = np.fft.fft(np.pad(w2.astype(np.float64), ((0, 8 - K), (0, 0), (0, 0))),
                    axis=0)
    Uc = np.conj(wf) / 8.0
    planes = [Uc[0].real, Uc[4].real]
    for j in (1, 2, 3):
        planes += [2 * Uc[j].real, 2 * Uc[j].imag, -2 * Uc[j].real]
    ud = np.stack(planes)                      # (NU, 2C_in, 2C_out)
    ud = ud.reshape(NU, OB, P, OB, P)          # (u, ib, p, ob, o)
    ud = np.ascontiguousarray(ud.transpose(3, 2, 0, 1, 4))  # (ob, p, u, ib, o)
    ud = ud.astype(bf)

    b1d = np.ascontiguousarray(b1f.reshape(OB, P).T)
    b2d = np.ascontiguousarray(b2.reshape(OB, P).T)
    b3d = np.ascontiguousarray(b3f.reshape(1, C)).astype(bf)

    shared = {"w1t": w1d, "ut": ud, "w3t": w3d, "b1": b1d, "b2": b2d, "b3": b3d}
    in_maps = []
    for c in range(NCORES):
        m = dict(shared)
        xc = x[c * BLOC:(c + 1) * BLOC]
        # host t-permutation: block row 4u+j -> partition j*32+u
        xp = xc.reshape(BLOC, 8, 32, 4, C).transpose(0, 1, 3, 2, 4)
        m["xs"] = np.ascontiguousarray(xp.reshape(BLOC, T, C)).astype(bf)
        in_maps.append(m)
    return in_maps


def unpermute_out(o):
    # device row q=j*32+u of each 128-block holds t-offset 4u+j
    return np.ascontiguousarray(
        o.reshape(BLOC, 8, 4, 32, C).transpose(0, 1, 3, 2, 4).reshape(BLOC, T, C))


_NC = None
LAST_RESULTS = None


def kernel(**inputs) -> np.ndarray:
    global _NC, LAST_RESULTS
    if _NC is None:
        _NC = build_nc()
    in_maps = prepare_inputs(**inputs)
    res = run_bass_kernel_spmd(_NC, in_maps, list(range(NCORES)))
    LAST_RESULTS = res
    return np.concatenate([unpermute_out(r["out"]) for r in res.results], axis=0)
